# revision 13
# baseline (speedup 1.0000x reference)
"""Multi-head attention (B=2, T=4096, D=768, H=12) as a Bass/Tile kernel
for 8 Trainium2 NeuronCores.

Sharding: cores 0-3 own batch 0, cores 4-7 own batch 1; each core owns 3
heads. Host folds all bias constants (b_o and the b_v @ W_o terms) into a
single per-batch row added after the cross-core partial-sum gather.

Per-core pipeline:
  A) x^T arrives bf16. Q^T/K^T projections run bf16 (W stationary, x^T
     moving); the PSUM->SBUF conversion (ACT, Identity+bias) adds
     b_q/b_k and quantizes straight to fp8 e4m3. V stays bf16 in
     per-key-chunk V_aug tiles [128, 3*65] whose per-head 65th column is
     1.0.
  B) The attention streams 256-query blocks. scores^T[k, q] =
     K^T-chunk.T @ Q^T as fp8 DoubleRow matmuls (the second k-tile of
     the pair points at a zeroed column range, so the product is
     unchanged); the three heads of a key chunk land in one 768-wide
     slot of a manually-rotated 6-bank PSUM region (depth-4 pipeline)
     and take ONE exp op. exp alternates between ACT (true exp, scale
     fused, bf16 out) and DVE (one-op Schraudolph: the fp32 affine
     s*AS + BS rounds to an integer whose low half-word IS the bf16 bit
     pattern of ~exp(s/8); read back via a stride-2 bf16 view).
  C) attn[q, 65]_h accumulates exp-chunk.T @ V_aug over 32 key chunks
     (moving is the 65-wide V_aug; column 64 yields sumexp[q] per
     partition; one PSUM bank per block). A single DVE tensor_tensor
     with a stride-0-broadcast 1/sumexp view normalizes [q, 195] to
     bf16; two XBAR DMA transposes produce the [a, q] stationaries for
     the W_o projection (output into a stolen score slot); the
     normalized aug columns == 1 land on zeroed W_o rows.
"""
import sys
import os
import numpy as np

try:
    import jax
    jax.config.update("jax_compilation_cache_dir", "/tmp/jax_cache_mha")
    jax.config.update("jax_persistent_cache_min_compile_time_secs", 1.0)
except Exception:
    pass

if "/opt/trn_rl_repo" not in sys.path:
    sys.path.insert(0, "/opt/trn_rl_repo")

N_CORES = 8
B, T, D, H, DK = 2, 4096, 768, 12, 64
HPC = 3           # heads per core
NKC = T // 128    # 32 key chunks
QB = 256          # queries per block
NQB = T // QB     # 16 query blocks
LAG = int(os.environ.get("K_LAG", "4"))
TAIL1_KC = int(os.environ.get("K_T1", "1"))
TAIL2_KC = int(os.environ.get("K_T2", "6"))
EBUF = int(os.environ.get("K_EBUF", "8"))
FP8 = int(os.environ.get("K_FP8", "1"))
ABL_NOT2 = int(os.environ.get("K_NOT2", "0"))   # ablation: skip O-proj tails
ABL_NOT1 = int(os.environ.get("K_NOT1", "0"))   # ablation: skip tail1 too
ABL_NOAV = int(os.environ.get("K_NOAV", "0"))   # ablation: skip attnV
NSLOT = 4         # rotating 768-wide score slots in the 6-bank PSUM region

# Per-slot-use exp engine: 'A' = ACT true exp, 'D' = DVE Schraudolph
EXP_PAT = os.environ.get("K_PAT", "ADADADADADADADADADA")

# Schraudolph: low half-word of fp32(s*AS + BS) is the bf16 bit pattern of
# exp(s*0.125)*(1+eps). AS = 0.125*128/ln2. BS scales by c = E[rho]/E[rho^2]
# (rho(f) = (1+f)/2^f), minimizing the RMS of eps: rms 1.8%, |eps| <= 4%.
_AS = 0.125 * 128.0 / np.log(2.0)
_F = np.linspace(0, 1, 200001)[:-1]
_RHO = (1 + _F) / np.exp2(_F)
_BS = 12582912.0 + 16256.0 + 128.0 * np.log2(_RHO.mean() / (_RHO ** 2).mean())

_cache = {}


def _build_nc():
    import concourse.bass as bass  # noqa: F401
    import concourse.mybir as mybir
    import concourse.tile as tile
    from concourse import bacc

    f32 = mybir.dt.float32
    bf16 = mybir.dt.bfloat16
    fp8 = mybir.dt.float8e4
    qk_dt = fp8 if FP8 else bf16
    AF = mybir.ActivationFunctionType
    ALU = mybir.AluOpType
    DR = mybir.MatmulPerfMode.DoubleRow

    nc = bacc.Bacc(None, target_bir_lowering=False)
    xbT = nc.dram_tensor("xbT", [D, T], bf16, kind="ExternalInput")
    wqk = nc.dram_tensor("wqk", [D, 384], bf16, kind="ExternalInput")
    wv = nc.dram_tensor("wv", [D, 192], bf16, kind="ExternalInput")
    wo1 = nc.dram_tensor("wo1", [128, D], bf16, kind="ExternalInput")
    wo2 = nc.dram_tensor("wo2", [67, D], bf16, kind="ExternalInput")
    bpack = nc.dram_tensor("bpack", [128, 3], f32, kind="ExternalInput")
    o = nc.dram_tensor("o", [T, D], f32, kind="ExternalOutput")

    QW = 2 * T if FP8 else T  # Q/K tile width (fp8 keeps a zeroed 2nd half)

    with tile.TileContext(nc) as tc:
        with tc.tile_pool(name="pers", bufs=1) as pers, \
             tc.tile_pool(name="expp", bufs=EBUF) as expp, \
             tc.tile_pool(name="attn", bufs=4) as attnp, \
             tc.tile_pool(name="accp", bufs=2, space="PSUM") as accp, \
             tc.tile_pool(name="scp", bufs=3, space="PSUM") as scp:

            # ---------------- persistent SBUF ----------------
            wqk_t = pers.tile([128, 6 * 384], bf16, tag="wqk")
            nc.sync.dma_start(
                out=wqk_t.rearrange("p (a c) -> p a c", a=6),
                in_=wqk[:, :].rearrange("(a p) c -> p a c", p=128))
            wv_t = pers.tile([128, 6 * 192], bf16, tag="wv")
            nc.sync.dma_start(
                out=wv_t.rearrange("p (a c) -> p a c", a=6),
                in_=wv[:, :].rearrange("(a p) c -> p a c", p=128))
            wo1_t = pers.tile([128, D], bf16, tag="wo1")
            nc.sync.dma_start(out=wo1_t, in_=wo1[:, :])
            wo2_t = pers.tile([67, D], bf16, tag="wo2")
            nc.sync.dma_start(out=wo2_t, in_=wo2[:, :])
            bias_t = pers.tile([128, 3], f32, tag="bias")
            nc.sync.dma_start(out=bias_t, in_=bpack[:, :])

            xt = [pers.tile([128, T], bf16, tag=f"xt{dc}", name=f"xt{dc}")
                  for dc in range(6)]
            for dc in range(6):
                nc.sync.dma_start(out=xt[dc],
                                  in_=xbT[dc * 128:(dc + 1) * 128, :])

            # Q/K tiles (fp8 or bf16); fp8 keeps cols T..2T zeroed for the
            # DoubleRow dummy second k-tile.
            qA = pers.tile([128, QW], qk_dt, tag="qA")
            kA = pers.tile([128, QW], qk_dt, tag="kA")
            qB = pers.tile([64, QW], qk_dt, tag="qB")
            k2s = pers.tile([128, QW], qk_dt, tag="k2s")  # rows 64:128 used
            kB = pers.tile([64, QW], qk_dt, tag="kB")
            if FP8:
                for t_ in (qA, kA, qB, kB):
                    nc.gpsimd.memset(t_[:, T:2 * T], 0.0)

            # V_aug: per key chunk [128, 3*65] bf16, col 65h+64 = 1.0
            vaug = pers.tile([128, NKC * 195], bf16, tag="vaug")
            vaug4 = vaug.rearrange("p (k h c) -> p k h c", k=NKC, h=3)
            nc.gpsimd.memset(vaug4[:, :, :, 64], 1.0)

            def next_slot():
                return scp.tile([128, 768], f32, tag="sc", name="sc")

            exp_tiles = {}   # (b, kc) -> (kind, tile)
            tailst = {}      # (b, q2) -> (aT1, aT2)

            def qk_ap(t_, rows, cs):
                """[rows, 2, len(cs)] AP: k-tile pair (data, zeros)."""
                return t_.rearrange("p (j c) -> p j c", j=2)[rows, :, cs]

            # ---------------- emit helpers ----------------
            def emit_scores_exp(b, kc):
                qs = slice(b * QB, (b + 1) * QB)
                ks = slice(kc * 128, (kc + 1) * 128)
                sc = next_slot()
                if FP8:
                    mm = [(sc[:, 0:256], qk_ap(kA, slice(0, 64), ks),
                           qk_ap(qA, slice(0, 64), qs), None),
                          (sc[:, 256:512], qk_ap(kA, slice(64, 128), ks),
                           qk_ap(qA, slice(64, 128), qs), (64, 0)),
                          (sc[:, 512:768], qk_ap(kB, slice(0, 64), ks),
                           qk_ap(qB, slice(0, 64), qs), None)]
                    for out_, l_, r_, tp in mm:
                        nc.tensor.matmul(out_, l_, r_, perf_mode=DR,
                                         start=True, stop=True,
                                         tile_position=tp,
                                         skip_group_check=True)
                else:
                    mm = [(sc[:, 0:256], kA[0:64, ks], qA[0:64, qs], None),
                          (sc[:, 256:512], kA[64:128, ks],
                           qA[64:128, qs], (64, 0)),
                          (sc[:, 512:768], kB[:, ks], qB[:, qs], None)]
                    for out_, l_, r_, tp in mm:
                        nc.tensor.matmul(out_, l_, r_, start=True, stop=True,
                                         tile_position=tp,
                                         skip_group_check=True)
                eng = EXP_PAT[(b * NKC + kc) % len(EXP_PAT)]
                if eng == "A":
                    e = expp.tile([128, 768], bf16, tag="ea", name="ea")
                    nc.scalar.activation(e, sc, AF.Exp, scale=0.125)
                else:
                    e = expp.tile([128, 768], f32, tag="eb", name="eb")
                    nc.vector.tensor_scalar(e, sc, float(_AS), float(_BS),
                                            ALU.mult, ALU.add)
                exp_tiles[(b, kc)] = (eng, e)

            def emit_attnv(b, kc, acc):
                eng, e = exp_tiles.pop((b, kc))
                if ABL_NOAV:
                    return
                if eng == "A":
                    full = e
                else:
                    full = e.bitcast(bf16).rearrange(
                        "p (c x) -> p c x", x=2)[:, :, 0]
                for h in range(HPC):
                    for q2 in range(2):
                        stat = full[:, h * 256 + q2 * 128:
                                    h * 256 + q2 * 128 + 128]
                        off = q2 * 195 + h * 65
                        nc.tensor.matmul(
                            acc[:, off:off + 65], stat,
                            vaug[:, kc * 195 + h * 65:kc * 195 + h * 65 + 65],
                            start=(kc == 0 and h == 0 and q2 == 0),
                            stop=(kc == NKC - 1 and h == HPC - 1 and q2 == 1),
                            skip_group_check=True)

            def emit_tail1(b, acc):
                """recip + stride-0-broadcast normalize (bf16) + XBAR."""
                if ABL_NOT1 or ABL_NOAV:
                    return
                for q2 in range(2):
                    off = q2 * 195
                    rc = attnp.tile([128, 4], f32, tag="rc", name="rc")
                    se = acc[:, off:off + 195].rearrange(
                        "p (c x) -> p c x", x=65)[:, :, 64]
                    nc.vector.reciprocal(rc[:, 0:3], se)
                    an = attnp.tile([128, 256], bf16, tag="an", name="an")
                    rcb = rc[:, 0:3].unsqueeze(2).broadcast_to([128, 3, 65])
                    nc.vector.tensor_tensor(
                        an[:, 0:195].rearrange("p (h c) -> p h c", h=3),
                        acc[:, off:off + 195].rearrange(
                            "p (h c) -> p h c", h=3),
                        rcb, ALU.mult)
                    nc.gpsimd.memset(an[:, 195:256], 0.0)
                    aT1 = attnp.tile([128, 128], bf16, tag="aT1", name="aT1")
                    aT2 = attnp.tile([128, 128], bf16, tag="aT2", name="aT2")
                    nc.sync.dma_start_transpose(aT1, an[:, 0:128])
                    nc.sync.dma_start_transpose(aT2, an[:, 128:256])
                    tailst[(b, q2)] = (aT1, aT2)

            def emit_tail2(b):
                """O-projection into a stolen score slot + store."""
                if ABL_NOT2 or ABL_NOT1 or ABL_NOAV:
                    return
                for q2 in range(2):
                    aT1, aT2 = tailst.pop((b, q2))
                    op = next_slot()
                    for third in range(3):
                        oc = slice(third * 256, third * 256 + 256)
                        nc.tensor.matmul(op[:, oc], aT1, wo1_t[:, oc],
                                         start=True, stop=False,
                                         skip_group_check=True)
                        nc.tensor.matmul(op[:, oc], aT2[0:67, :],
                                         wo2_t[:, oc], start=False, stop=True,
                                         skip_group_check=True)
                    ot = attnp.tile([128, D], f32, tag="ot", name="ot")
                    nc.scalar.activation(ot, op, AF.Copy)
                    r0 = b * QB + q2 * 128
                    nc.sync.dma_start(out=o[r0:r0 + 128, :], in_=ot)

            # ---------------- phase A (+ block-0 interleave) ----------------
            def emit_proj(t):
                tcols = slice(t * 512, (t + 1) * 512)
                for g in range(3):
                    pj = next_slot()
                    for dc in range(6):
                        nc.tensor.matmul(
                            pj[:, 0:512],
                            wqk_t[:, dc * 384 + g * 128:
                                  dc * 384 + (g + 1) * 128],
                            xt[dc][:, tcols], start=(dc == 0),
                            stop=(dc == 5), skip_group_check=True)
                    if g == 0:
                        nc.scalar.activation(qA[:, tcols], pj[:, 0:512],
                                             AF.Identity,
                                             bias=bias_t[:, 0:1])
                    elif g == 1:
                        nc.scalar.activation(kA[:, tcols], pj[:, 0:512],
                                             AF.Identity,
                                             bias=bias_t[:, 1:2])
                    else:
                        nc.scalar.activation(qB[:, tcols], pj[0:64, 0:512],
                                             AF.Identity,
                                             bias=bias_t[0:64, 2:3])
                        nc.scalar.activation(k2s[64:128, tcols],
                                             pj[64:128, 0:512],
                                             AF.Identity,
                                             bias=bias_t[64:128, 2:3])
                if t % 2 == 1:
                    sh = slice((t - 1) * 512, (t + 1) * 512)
                    nc.sync.dma_start(out=kB[:, sh], in_=k2s[64:128, sh])
                for i in range(4):
                    kc = t * 4 + i
                    vp = next_slot()
                    for dc in range(6):
                        nc.tensor.matmul(
                            vp[:, 0:192],
                            xt[dc][:, kc * 128:(kc + 1) * 128],
                            wv_t[:, dc * 192:(dc + 1) * 192],
                            start=(dc == 0), stop=(dc == 5),
                            skip_group_check=True)
                    nc.scalar.activation(
                        vaug4[:, kc, :, 0:64],
                        vp[:, 0:192].rearrange("p (h c) -> p h c", h=3),
                        AF.Copy)

            b0_acc = accp.tile([128, 390], f32, tag="acc", name="acc0")
            for t in range(8):
                emit_proj(t)
                if t >= 1:
                    for kc in range(4 * (t - 1), 4 * t):
                        emit_scores_exp(0, kc)
                        if kc >= LAG:
                            emit_attnv(0, kc - LAG, b0_acc)

            # ---------------- blocks ----------------
            carry = None
            cur = (0, b0_acc)
            for b in range(NQB):
                if b == 0:
                    for kc in range(28, 32):
                        emit_scores_exp(0, kc)
                        emit_attnv(0, kc - LAG, b0_acc)
                    for kc in range(NKC - LAG, NKC):
                        emit_attnv(0, kc, b0_acc)
                else:
                    acc = accp.tile([128, 390], f32, tag="acc", name="acc")
                    cur = (b, acc)
                    for kc in range(NKC):
                        emit_scores_exp(b, kc)
                        if kc >= LAG:
                            emit_attnv(b, kc - LAG, acc)
                        if kc == TAIL1_KC and carry is not None:
                            emit_tail1(carry[0], carry[1])
                        if kc == TAIL2_KC and carry is not None:
                            emit_tail2(carry[0])
                    for kc in range(NKC - LAG, NKC):
                        emit_attnv(b, kc, acc)
                carry = cur
            emit_tail1(carry[0], carry[1])
            emit_tail2(carry[0])

    nc.finalize()
    return nc


def _get_nc():
    if "nc" not in _cache:
        _cache["nc"] = _build_nc()
    return _cache["nc"]


def _make_in_maps(x, W_q, b_q, W_k, b_k, W_v, b_v, W_o, b_o):
    import ml_dtypes
    bf = ml_dtypes.bfloat16
    in_maps = []
    for c in range(N_CORES):
        b = c // 4
        h0 = (c % 4) * HPC
        c0 = h0 * DK

        g0 = W_q[:, c0:c0 + 128]
        g1 = W_k[:, c0:c0 + 128]
        g2 = np.concatenate([W_q[:, c0 + 128:c0 + 192],
                             W_k[:, c0 + 128:c0 + 192]], axis=1)
        wqk_m = np.concatenate([g0, g1, g2], axis=1)

        bpack = np.zeros((128, 3), np.float32)
        bpack[:, 0] = b_q[c0:c0 + 128]
        bpack[:, 1] = b_k[c0:c0 + 128]
        bpack[0:64, 2] = b_q[c0 + 128:c0 + 192]
        bpack[64:128, 2] = b_k[c0 + 128:c0 + 192]

        # woaug [195, 768]: per head rows 0..63 = W_o rows; row 64 = 0
        woaug = np.zeros((195, D), np.float32)
        for j in range(HPC):
            woaug[j * 65:j * 65 + 64, :] = \
                W_o[c0 + j * DK:c0 + (j + 1) * DK, :]

        in_maps.append({
            "xbT": np.ascontiguousarray(x[b].T).astype(bf),
            "wqk": np.ascontiguousarray(wqk_m).astype(bf),
            "wv": np.ascontiguousarray(W_v[:, c0:c0 + 192]).astype(bf),
            "wo1": np.ascontiguousarray(woaug[0:128, :]).astype(bf),
            "wo2": np.ascontiguousarray(woaug[128:195, :]).astype(bf),
            "bpack": bpack,
        })
    return in_maps


def kernel(**inputs):
    from concourse.bass_utils import run_bass_kernel_spmd

    args = {k: np.asarray(v, dtype=np.float32) for k, v in inputs.items()}
    in_maps = _make_in_maps(
        args["x"], args["W_q"], args["b_q"], args["W_k"], args["b_k"],
        args["W_v"], args["b_v"], args["W_o"], args["b_o"])

    nc = _get_nc()
    trace = bool(int(os.environ.get("KBENCH_TRACE", "0")))
    res = run_bass_kernel_spmd(nc, in_maps, core_ids=list(range(N_CORES)),
                               trace=trace)
    _cache["last_result"] = res

    out = np.zeros((B, T, D), np.float32)
    for c in range(N_CORES):
        out[c // 4] += res.results[c]["o"]
    # bias constants folded on host: b_o plus every head's b_v @ W_o
    bias_row = args["b_o"] + args["b_v"] @ args["W_o"]
    out += bias_row[None, None, :]
    return out


# revision 14
# speedup vs baseline: 1.1070x; 1.1070x over previous
"""Multi-head attention (B=2, T=4096, D=768, H=12) as a Bass/Tile kernel
for 8 Trainium2 NeuronCores.

Sharding: cores 0-3 own batch 0, cores 4-7 own batch 1; each core owns 3
heads. Host folds all bias constants (b_o and the b_v @ W_o terms) into a
single per-batch row added after the cross-core partial-sum gather.

Per-core pipeline:
  A) x^T arrives bf16. Q^T/K^T projections run bf16 (W stationary, x^T
     moving); the PSUM->SBUF conversion (ACT, Identity+bias) adds
     b_q/b_k and quantizes straight to fp8 e4m3. V stays bf16 in
     per-key-chunk V_aug tiles [128, 3*65] whose per-head 65th column is
     1.0.
  B) The attention streams 256-query blocks. scores^T[k, q] =
     K^T-chunk.T @ Q^T as fp8 DoubleRow matmuls (the second k-tile of
     the pair points at a zeroed column range, so the product is
     unchanged); the three heads of a key chunk land in one 768-wide
     slot of a manually-rotated 6-bank PSUM region (depth-4 pipeline)
     and take ONE exp op. exp alternates between ACT (true exp, scale
     fused, bf16 out) and DVE (one-op Schraudolph: the fp32 affine
     s*AS + BS rounds to an integer whose low half-word IS the bf16 bit
     pattern of ~exp(s/8); read back via a stride-2 bf16 view).
  C) attn[q, 65]_h accumulates exp-chunk.T @ V_aug over 32 key chunks
     (moving is the 65-wide V_aug; column 64 yields sumexp[q] per
     partition; one PSUM bank per block). A single DVE tensor_tensor
     with a stride-0-broadcast 1/sumexp view normalizes [q, 195] to
     bf16; two XBAR DMA transposes produce the [a, q] stationaries for
     the W_o projection (output into a stolen score slot); the
     normalized aug columns == 1 land on zeroed W_o rows.
"""
import sys
import os
import numpy as np

try:
    import jax
    jax.config.update("jax_compilation_cache_dir", "/tmp/jax_cache_mha")
    jax.config.update("jax_persistent_cache_min_compile_time_secs", 1.0)
except Exception:
    pass

if "/opt/trn_rl_repo" not in sys.path:
    sys.path.insert(0, "/opt/trn_rl_repo")

N_CORES = 8
B, T, D, H, DK = 2, 4096, 768, 12, 64
HPC = 3           # heads per core
NKC = T // 128    # 32 key chunks
QB = 256          # queries per block
NQB = T // QB     # 16 query blocks
LAG = int(os.environ.get("K_LAG", "4"))
TAIL1_KC = int(os.environ.get("K_T1", "1"))
TAIL2_KC = int(os.environ.get("K_T2", "6"))
EBUF = int(os.environ.get("K_EBUF", "8"))
FP8 = int(os.environ.get("K_FP8", "1"))
ABL_NOT2 = int(os.environ.get("K_NOT2", "0"))   # ablation: skip O-proj tails
ABL_NOT1 = int(os.environ.get("K_NOT1", "0"))   # ablation: skip tail1 too
ABL_NOAV = int(os.environ.get("K_NOAV", "0"))   # ablation: skip attnV
NSLOT = 4         # rotating 768-wide score slots in the 6-bank PSUM region

# Per-slot-use exp engine: 'A' = ACT true exp, 'D' = DVE Schraudolph
EXP_PAT = os.environ.get("K_PAT", "ADADADADADADADADADA")

# Schraudolph: low half-word of fp32(s*AS + BS) is the bf16 bit pattern of
# exp(s*0.125)*(1+eps). AS = 0.125*128/ln2. BS scales by c = E[rho]/E[rho^2]
# (rho(f) = (1+f)/2^f), minimizing the RMS of eps: rms 1.8%, |eps| <= 4%.
_AS = 0.125 * 128.0 / np.log(2.0)
_F = np.linspace(0, 1, 200001)[:-1]
_RHO = (1 + _F) / np.exp2(_F)
_BS = 12582912.0 + 16256.0 + 128.0 * np.log2(_RHO.mean() / (_RHO ** 2).mean())

_cache = {}


def _build_nc():
    import concourse.bass as bass  # noqa: F401
    import concourse.mybir as mybir
    import concourse.tile as tile
    from concourse import bacc

    f32 = mybir.dt.float32
    bf16 = mybir.dt.bfloat16
    fp8 = mybir.dt.float8e4
    qk_dt = fp8 if FP8 else bf16
    AF = mybir.ActivationFunctionType
    ALU = mybir.AluOpType
    DR = mybir.MatmulPerfMode.DoubleRow

    nc = bacc.Bacc(None, target_bir_lowering=False)
    xbT = nc.dram_tensor("xbT", [D, T], bf16, kind="ExternalInput")
    wqk = nc.dram_tensor("wqk", [D, 384], bf16, kind="ExternalInput")
    wv = nc.dram_tensor("wv", [D, 192], bf16, kind="ExternalInput")
    wo1 = nc.dram_tensor("wo1", [128, D], bf16, kind="ExternalInput")
    wo2 = nc.dram_tensor("wo2", [67, D], bf16, kind="ExternalInput")
    bpack = nc.dram_tensor("bpack", [128, 3], f32, kind="ExternalInput")
    o = nc.dram_tensor("o", [T, D], f32, kind="ExternalOutput")

    QW = 2 * T if FP8 else T  # Q/K tile width (fp8 keeps a zeroed 2nd half)

    with tile.TileContext(nc) as tc:
        with tc.tile_pool(name="pers", bufs=1) as pers, \
             tc.tile_pool(name="expp", bufs=EBUF) as expp, \
             tc.tile_pool(name="attn", bufs=4) as attnp, \
             tc.tile_pool(name="accp", bufs=2, space="PSUM") as accp, \
             tc.tile_pool(name="scp", bufs=3, space="PSUM") as scp:

            # ---------------- persistent SBUF ----------------
            wqk_t = pers.tile([128, 6 * 384], bf16, tag="wqk")
            nc.sync.dma_start(
                out=wqk_t.rearrange("p (a c) -> p a c", a=6),
                in_=wqk[:, :].rearrange("(a p) c -> p a c", p=128))
            wv_t = pers.tile([128, 6 * 192], bf16, tag="wv")
            nc.sync.dma_start(
                out=wv_t.rearrange("p (a c) -> p a c", a=6),
                in_=wv[:, :].rearrange("(a p) c -> p a c", p=128))
            wo1_t = pers.tile([128, D], bf16, tag="wo1")
            nc.sync.dma_start(out=wo1_t, in_=wo1[:, :])
            wo2_t = pers.tile([67, D], bf16, tag="wo2")
            nc.sync.dma_start(out=wo2_t, in_=wo2[:, :])
            bias_t = pers.tile([128, 3], f32, tag="bias")
            nc.sync.dma_start(out=bias_t, in_=bpack[:, :])

            xt = [pers.tile([128, T], bf16, tag=f"xt{dc}", name=f"xt{dc}")
                  for dc in range(6)]
            for dc in range(6):
                nc.sync.dma_start(out=xt[dc],
                                  in_=xbT[dc * 128:(dc + 1) * 128, :])

            # Q/K tiles (fp8 or bf16); fp8 keeps cols T..2T zeroed for the
            # DoubleRow dummy second k-tile.
            qA = pers.tile([128, QW], qk_dt, tag="qA")
            kA = pers.tile([128, QW], qk_dt, tag="kA")
            qB = pers.tile([64, QW], qk_dt, tag="qB")
            k2s = pers.tile([128, QW], qk_dt, tag="k2s")  # rows 64:128 used
            kB = pers.tile([64, QW], qk_dt, tag="kB")
            if FP8:
                for t_ in (qA, kA, qB, kB):
                    nc.gpsimd.memset(t_[:, T:2 * T], 0.0)

            # V_aug: per key chunk [128, 3*65] bf16, col 65h+64 = 1.0
            vaug = pers.tile([128, NKC * 195], bf16, tag="vaug")
            vaug4 = vaug.rearrange("p (k h c) -> p k h c", k=NKC, h=3)
            nc.gpsimd.memset(vaug4[:, :, :, 64], 1.0)

            def next_slot():
                return scp.tile([128, 768], f32, tag="sc", name="sc")

            exp_tiles = {}   # (b, kc) -> (kind, tile)
            tailst = {}      # (b, q2) -> (aT1, aT2)

            def qk_ap(t_, rows, cs):
                """[rows, 2, len(cs)] AP: k-tile pair (data, zeros)."""
                return t_.rearrange("p (j c) -> p j c", j=2)[rows, :, cs]

            # ---------------- emit helpers ----------------
            def emit_scores_exp(b, kc):
                qs = slice(b * QB, (b + 1) * QB)
                ks = slice(kc * 128, (kc + 1) * 128)
                sc = next_slot()
                if FP8:
                    mm = [(sc[:, 0:256], qk_ap(kA, slice(0, 64), ks),
                           qk_ap(qA, slice(0, 64), qs), None),
                          (sc[:, 256:512], qk_ap(kA, slice(64, 128), ks),
                           qk_ap(qA, slice(64, 128), qs), (64, 0)),
                          (sc[:, 512:768], qk_ap(kB, slice(0, 64), ks),
                           qk_ap(qB, slice(0, 64), qs), None)]
                    for out_, l_, r_, tp in mm:
                        nc.tensor.matmul(out_, l_, r_, perf_mode=DR,
                                         start=True, stop=True,
                                         tile_position=tp,
                                         skip_group_check=True)
                else:
                    mm = [(sc[:, 0:256], kA[0:64, ks], qA[0:64, qs], None),
                          (sc[:, 256:512], kA[64:128, ks],
                           qA[64:128, qs], (64, 0)),
                          (sc[:, 512:768], kB[:, ks], qB[:, qs], None)]
                    for out_, l_, r_, tp in mm:
                        nc.tensor.matmul(out_, l_, r_, start=True, stop=True,
                                         tile_position=tp,
                                         skip_group_check=True)
                eng = EXP_PAT[(b * NKC + kc) % len(EXP_PAT)]
                if eng == "A":
                    e = expp.tile([128, 768], bf16, tag="ea", name="ea")
                    nc.scalar.activation(e, sc, AF.Exp, scale=0.125)
                else:
                    e = expp.tile([128, 768], f32, tag="eb", name="eb")
                    nc.vector.tensor_scalar(e, sc, float(_AS), float(_BS),
                                            ALU.mult, ALU.add)
                exp_tiles[(b, kc)] = (eng, e)

            def emit_attnv(b, kc, acc):
                eng, e = exp_tiles.pop((b, kc))
                if ABL_NOAV:
                    return
                if eng == "A":
                    full = e
                else:
                    full = e.bitcast(bf16).rearrange(
                        "p (c x) -> p c x", x=2)[:, :, 0]
                for h in range(HPC):
                    for q2 in range(2):
                        stat = full[:, h * 256 + q2 * 128:
                                    h * 256 + q2 * 128 + 128]
                        off = q2 * 195 + h * 65
                        nc.tensor.matmul(
                            acc[:, off:off + 65], stat,
                            vaug[:, kc * 195 + h * 65:kc * 195 + h * 65 + 65],
                            start=(kc == 0 and h == 0 and q2 == 0),
                            stop=(kc == NKC - 1 and h == HPC - 1 and q2 == 1),
                            skip_group_check=True)

            def emit_tail1(b, acc):
                """recip + stride-0-broadcast normalize (bf16) + XBAR."""
                if ABL_NOT1 or ABL_NOAV:
                    return
                for q2 in range(2):
                    off = q2 * 195
                    rc = attnp.tile([128, 4], f32, tag="rc", name="rc")
                    se = acc[:, off:off + 195].rearrange(
                        "p (c x) -> p c x", x=65)[:, :, 64]
                    nc.vector.reciprocal(rc[:, 0:3], se)
                    an = attnp.tile([128, 256], bf16, tag="an", name="an")
                    rcb = rc[:, 0:3].unsqueeze(2).broadcast_to([128, 3, 65])
                    nc.vector.tensor_tensor(
                        an[:, 0:195].rearrange("p (h c) -> p h c", h=3),
                        acc[:, off:off + 195].rearrange(
                            "p (h c) -> p h c", h=3),
                        rcb, ALU.mult)
                    nc.gpsimd.memset(an[:, 195:256], 0.0)
                    aT1 = attnp.tile([128, 128], bf16, tag="aT1", name="aT1")
                    aT2 = attnp.tile([128, 128], bf16, tag="aT2", name="aT2")
                    nc.sync.dma_start_transpose(aT1, an[:, 0:128])
                    nc.sync.dma_start_transpose(aT2, an[:, 128:256])
                    tailst[(b, q2)] = (aT1, aT2)

            def emit_tail2(b):
                """O-projection into a stolen score slot + store."""
                if ABL_NOT2 or ABL_NOT1 or ABL_NOAV:
                    return
                for q2 in range(2):
                    aT1, aT2 = tailst.pop((b, q2))
                    op = next_slot()
                    for third in range(3):
                        oc = slice(third * 256, third * 256 + 256)
                        nc.tensor.matmul(op[:, oc], aT1, wo1_t[:, oc],
                                         start=True, stop=False,
                                         skip_group_check=True)
                        nc.tensor.matmul(op[:, oc], aT2[0:67, :],
                                         wo2_t[:, oc], start=False, stop=True,
                                         skip_group_check=True)
                    ot = attnp.tile([128, D], f32, tag="ot", name="ot")
                    nc.scalar.activation(ot, op, AF.Copy)
                    r0 = b * QB + q2 * 128
                    nc.sync.dma_start(out=o[r0:r0 + 128, :], in_=ot)

            # ---------------- phase A (+ block-0 interleave) ----------------
            def emit_proj(t):
                tcols = slice(t * 512, (t + 1) * 512)
                for g in range(3):
                    pj = next_slot()
                    for dc in range(6):
                        nc.tensor.matmul(
                            pj[:, 0:512],
                            wqk_t[:, dc * 384 + g * 128:
                                  dc * 384 + (g + 1) * 128],
                            xt[dc][:, tcols], start=(dc == 0),
                            stop=(dc == 5), skip_group_check=True)
                    if g == 0:
                        nc.scalar.activation(qA[:, tcols], pj[:, 0:512],
                                             AF.Identity,
                                             bias=bias_t[:, 0:1])
                    elif g == 1:
                        nc.scalar.activation(kA[:, tcols], pj[:, 0:512],
                                             AF.Identity,
                                             bias=bias_t[:, 1:2])
                    else:
                        nc.scalar.activation(qB[:, tcols], pj[0:64, 0:512],
                                             AF.Identity,
                                             bias=bias_t[0:64, 2:3])
                        nc.scalar.activation(k2s[64:128, tcols],
                                             pj[64:128, 0:512],
                                             AF.Identity,
                                             bias=bias_t[64:128, 2:3])
                if t % 2 == 1:
                    sh = slice((t - 1) * 512, (t + 1) * 512)
                    nc.sync.dma_start(out=kB[:, sh], in_=k2s[64:128, sh])
                for i in range(4):
                    kc = t * 4 + i
                    vp = next_slot()
                    for dc in range(6):
                        nc.tensor.matmul(
                            vp[:, 0:192],
                            xt[dc][:, kc * 128:(kc + 1) * 128],
                            wv_t[:, dc * 192:(dc + 1) * 192],
                            start=(dc == 0), stop=(dc == 5),
                            skip_group_check=True)
                    nc.vector.tensor_copy(
                        vaug4[:, kc, :, 0:64],
                        vp[:, 0:192].rearrange("p (h c) -> p h c", h=3))

            b0_acc = accp.tile([128, 390], f32, tag="acc", name="acc0")
            for t in range(8):
                emit_proj(t)
                if t >= 1:
                    for kc in range(4 * (t - 1), 4 * t):
                        emit_scores_exp(0, kc)
                        if kc >= LAG:
                            emit_attnv(0, kc - LAG, b0_acc)

            # ---------------- blocks ----------------
            carry = None
            cur = (0, b0_acc)
            for b in range(NQB):
                if b == 0:
                    for kc in range(28, 32):
                        emit_scores_exp(0, kc)
                        emit_attnv(0, kc - LAG, b0_acc)
                    for kc in range(NKC - LAG, NKC):
                        emit_attnv(0, kc, b0_acc)
                else:
                    acc = accp.tile([128, 390], f32, tag="acc", name="acc")
                    cur = (b, acc)
                    for kc in range(NKC):
                        emit_scores_exp(b, kc)
                        if kc >= LAG:
                            emit_attnv(b, kc - LAG, acc)
                        if kc == TAIL1_KC and carry is not None:
                            emit_tail1(carry[0], carry[1])
                        if kc == TAIL2_KC and carry is not None:
                            emit_tail2(carry[0])
                    for kc in range(NKC - LAG, NKC):
                        emit_attnv(b, kc, acc)
                carry = cur
            emit_tail1(carry[0], carry[1])
            emit_tail2(carry[0])

    nc.finalize()
    return nc


def _get_nc():
    if "nc" not in _cache:
        _cache["nc"] = _build_nc()
    return _cache["nc"]


def _make_in_maps(x, W_q, b_q, W_k, b_k, W_v, b_v, W_o, b_o):
    import ml_dtypes
    bf = ml_dtypes.bfloat16
    in_maps = []
    for c in range(N_CORES):
        b = c // 4
        h0 = (c % 4) * HPC
        c0 = h0 * DK

        g0 = W_q[:, c0:c0 + 128]
        g1 = W_k[:, c0:c0 + 128]
        g2 = np.concatenate([W_q[:, c0 + 128:c0 + 192],
                             W_k[:, c0 + 128:c0 + 192]], axis=1)
        wqk_m = np.concatenate([g0, g1, g2], axis=1)

        bpack = np.zeros((128, 3), np.float32)
        bpack[:, 0] = b_q[c0:c0 + 128]
        bpack[:, 1] = b_k[c0:c0 + 128]
        bpack[0:64, 2] = b_q[c0 + 128:c0 + 192]
        bpack[64:128, 2] = b_k[c0 + 128:c0 + 192]

        # woaug [195, 768]: per head rows 0..63 = W_o rows; row 64 = 0
        woaug = np.zeros((195, D), np.float32)
        for j in range(HPC):
            woaug[j * 65:j * 65 + 64, :] = \
                W_o[c0 + j * DK:c0 + (j + 1) * DK, :]

        in_maps.append({
            "xbT": np.ascontiguousarray(x[b].T).astype(bf),
            "wqk": np.ascontiguousarray(wqk_m).astype(bf),
            "wv": np.ascontiguousarray(W_v[:, c0:c0 + 192]).astype(bf),
            "wo1": np.ascontiguousarray(woaug[0:128, :]).astype(bf),
            "wo2": np.ascontiguousarray(woaug[128:195, :]).astype(bf),
            "bpack": bpack,
        })
    return in_maps


def kernel(**inputs):
    from concourse.bass_utils import run_bass_kernel_spmd

    args = {k: np.asarray(v, dtype=np.float32) for k, v in inputs.items()}
    in_maps = _make_in_maps(
        args["x"], args["W_q"], args["b_q"], args["W_k"], args["b_k"],
        args["W_v"], args["b_v"], args["W_o"], args["b_o"])

    nc = _get_nc()
    trace = bool(int(os.environ.get("KBENCH_TRACE", "0")))
    res = run_bass_kernel_spmd(nc, in_maps, core_ids=list(range(N_CORES)),
                               trace=trace)
    _cache["last_result"] = res

    out = np.zeros((B, T, D), np.float32)
    for c in range(N_CORES):
        out[c // 4] += res.results[c]["o"]
    # bias constants folded on host: b_o plus every head's b_v @ W_o
    bias_row = args["b_o"] + args["b_v"] @ args["W_o"]
    out += bias_row[None, None, :]
    return out


# revision 16
# speedup vs baseline: 1.1316x; 1.0223x over previous
"""Multi-head attention (B=2, T=4096, D=768, H=12) as a Bass/Tile kernel
for 8 Trainium2 NeuronCores.

Sharding: cores 0-3 own batch 0, cores 4-7 own batch 1; each core owns 3
heads. Host folds all bias constants (b_o and the b_v @ W_o terms) into a
single per-batch row added after the cross-core partial-sum gather.

Per-core pipeline:
  A) x^T arrives bf16. Q^T/K^T projections run bf16 (W stationary, x^T
     moving); the PSUM->SBUF conversion (ACT, Identity+bias) adds
     b_q/b_k and quantizes straight to fp8 e4m3. V stays bf16 in
     per-key-chunk V_aug tiles [128, 3*65] whose per-head 65th column is
     1.0.
  B) The attention streams 256-query blocks. scores^T[k, q] =
     K^T-chunk.T @ Q^T as fp8 DoubleRow matmuls (the second k-tile of
     the pair points at a zeroed column range, so the product is
     unchanged); the three heads of a key chunk land in one 768-wide
     slot of a manually-rotated 6-bank PSUM region (depth-4 pipeline)
     and take ONE exp op. exp alternates between ACT (true exp, scale
     fused, bf16 out) and DVE (one-op Schraudolph: the fp32 affine
     s*AS + BS rounds to an integer whose low half-word IS the bf16 bit
     pattern of ~exp(s/8); read back via a stride-2 bf16 view).
  C) attn[q, 65]_h accumulates exp-chunk.T @ V_aug over 32 key chunks
     (moving is the 65-wide V_aug; column 64 yields sumexp[q] per
     partition; one PSUM bank per block). A single DVE tensor_tensor
     with a stride-0-broadcast 1/sumexp view normalizes [q, 195] to
     bf16; two XBAR DMA transposes produce the [a, q] stationaries for
     the W_o projection (output into a stolen score slot); the
     normalized aug columns == 1 land on zeroed W_o rows.
"""
import sys
import os
import numpy as np

try:
    import jax
    jax.config.update("jax_compilation_cache_dir", "/tmp/jax_cache_mha")
    jax.config.update("jax_persistent_cache_min_compile_time_secs", 1.0)
except Exception:
    pass

if "/opt/trn_rl_repo" not in sys.path:
    sys.path.insert(0, "/opt/trn_rl_repo")

N_CORES = 8
B, T, D, H, DK = 2, 4096, 768, 12, 64
HPC = 3           # heads per core
NKC = T // 128    # 32 key chunks
QB = 256          # queries per block
NQB = T // QB     # 16 query blocks
LAG = int(os.environ.get("K_LAG", "4"))
TAIL1_KC = int(os.environ.get("K_T1", "1"))
TAIL2_KC = int(os.environ.get("K_T2", "14"))
EBUF = int(os.environ.get("K_EBUF", "8"))
FP8 = int(os.environ.get("K_FP8", "1"))
XP8 = int(os.environ.get("K_XP8", "1"))   # fp8 DoubleRow Q/K projections
WSC = 64.0                                # W_qk prescale (keeps fp8 normal)
ABL_NOT2 = int(os.environ.get("K_NOT2", "0"))   # ablation: skip O-proj tails
ABL_NOT1 = int(os.environ.get("K_NOT1", "0"))   # ablation: skip tail1 too
ABL_NOAV = int(os.environ.get("K_NOAV", "0"))   # ablation: skip attnV
NSLOT = 4         # rotating 768-wide score slots in the 6-bank PSUM region

# Per-slot-use exp engine: 'A' = ACT true exp, 'D' = DVE Schraudolph
EXP_PAT = os.environ.get("K_PAT", "AD")

# Schraudolph: low half-word of fp32(s*AS + BS) is the bf16 bit pattern of
# exp(s*0.125)*(1+eps). AS = 0.125*128/ln2. BS scales by c = E[rho]/E[rho^2]
# (rho(f) = (1+f)/2^f), minimizing the RMS of eps: rms 1.8%, |eps| <= 4%.
_AS = 0.125 * 128.0 / np.log(2.0)  # divided by 4096 at use when XP8
_F = np.linspace(0, 1, 200001)[:-1]
_RHO = (1 + _F) / np.exp2(_F)
_BS = 12582912.0 + 16256.0 + 128.0 * np.log2(_RHO.mean() / (_RHO ** 2).mean())

_cache = {}


def _build_nc():
    import concourse.bass as bass  # noqa: F401
    import concourse.mybir as mybir
    import concourse.tile as tile
    from concourse import bacc

    f32 = mybir.dt.float32
    bf16 = mybir.dt.bfloat16
    fp8 = mybir.dt.float8e4
    qk_dt = fp8 if FP8 else bf16
    AF = mybir.ActivationFunctionType
    ALU = mybir.AluOpType
    DR = mybir.MatmulPerfMode.DoubleRow

    nc = bacc.Bacc(None, target_bir_lowering=False)
    xbT = nc.dram_tensor("xbT", [D, T], bf16, kind="ExternalInput")
    wqk = nc.dram_tensor("wqk", [D, 384], bf16, kind="ExternalInput")
    x8 = nc.dram_tensor("x8", [384, 2 * T], fp8, kind="ExternalInput")
    wqk8 = nc.dram_tensor("wqk8", [128, 2304], fp8, kind="ExternalInput")
    wv = nc.dram_tensor("wv", [D, 192], bf16, kind="ExternalInput")
    wo1 = nc.dram_tensor("wo1", [128, D], bf16, kind="ExternalInput")
    wo2 = nc.dram_tensor("wo2", [67, D], bf16, kind="ExternalInput")
    bpack = nc.dram_tensor("bpack", [128, 3], f32, kind="ExternalInput")
    o = nc.dram_tensor("o", [T, D], f32, kind="ExternalOutput")

    QW = 2 * T if FP8 else T  # Q/K tile width (fp8 keeps a zeroed 2nd half)

    with tile.TileContext(nc) as tc:
        with tc.tile_pool(name="pers", bufs=1) as pers, \
             tc.tile_pool(name="expp", bufs=EBUF) as expp, \
             tc.tile_pool(name="attn", bufs=4) as attnp, \
             tc.tile_pool(name="accp", bufs=2, space="PSUM") as accp, \
             tc.tile_pool(name="scp", bufs=3, space="PSUM") as scp:

            # ---------------- persistent SBUF ----------------
            wqk_t = pers.tile([128, 6 * 384], bf16, tag="wqk")
            nc.sync.dma_start(
                out=wqk_t.rearrange("p (a c) -> p a c", a=6),
                in_=wqk[:, :].rearrange("(a p) c -> p a c", p=128))
            wv_t = pers.tile([128, 6 * 192], bf16, tag="wv")
            nc.sync.dma_start(
                out=wv_t.rearrange("p (a c) -> p a c", a=6),
                in_=wv[:, :].rearrange("(a p) c -> p a c", p=128))
            wo1_t = pers.tile([128, D], bf16, tag="wo1")
            nc.sync.dma_start(out=wo1_t, in_=wo1[:, :])
            wo2_t = pers.tile([67, D], bf16, tag="wo2")
            nc.sync.dma_start(out=wo2_t, in_=wo2[:, :])
            bias_t = pers.tile([128, 3], f32, tag="bias")
            nc.sync.dma_start(out=bias_t, in_=bpack[:, :])

            xt = [pers.tile([128, T], bf16, tag=f"xt{dc}", name=f"xt{dc}")
                  for dc in range(6)]
            for dc in range(6):
                nc.sync.dma_start(out=xt[dc],
                                  in_=xbT[dc * 128:(dc + 1) * 128, :])
            if XP8:
                x8t = [pers.tile([128, 2 * T], fp8, tag=f"x8{j}",
                                 name=f"x8{j}") for j in range(3)]
                for j in range(3):
                    nc.sync.dma_start(out=x8t[j],
                                      in_=x8[j * 128:(j + 1) * 128, :])
                wqk8_t = pers.tile([128, 2304], fp8, tag="wqk8")
                nc.sync.dma_start(out=wqk8_t, in_=wqk8[:, :])
                wqk8v = wqk8_t.rearrange("p (j g jj c) -> p j g jj c",
                                         j=3, g=3, jj=2)

            # Q/K tiles (fp8 or bf16); fp8 keeps cols T..2T zeroed for the
            # DoubleRow dummy second k-tile.
            qA = pers.tile([128, QW], qk_dt, tag="qA")
            kA = pers.tile([128, QW], qk_dt, tag="kA")
            qB = pers.tile([64, QW], qk_dt, tag="qB")
            k2s = pers.tile([128, QW], qk_dt, tag="k2s")  # rows 64:128 used
            kB = pers.tile([64, QW], qk_dt, tag="kB")
            if FP8:
                for t_ in (qA, kA, qB, kB):
                    nc.gpsimd.memset(t_[:, T:2 * T], 0.0)

            # V_aug: per key chunk [128, 3*65] bf16, col 65h+64 = 1.0
            vaug = pers.tile([128, NKC * 195], bf16, tag="vaug")
            vaug4 = vaug.rearrange("p (k h c) -> p k h c", k=NKC, h=3)
            nc.gpsimd.memset(vaug4[:, :, :, 64], 1.0)

            def next_slot():
                return scp.tile([128, 768], f32, tag="sc", name="sc")

            exp_tiles = {}   # (b, kc) -> (kind, tile)
            tailst = {}      # (b, q2) -> (aT1, aT2)

            def qk_ap(t_, rows, cs):
                """[rows, 2, len(cs)] AP: k-tile pair (data, zeros)."""
                return t_.rearrange("p (j c) -> p j c", j=2)[rows, :, cs]

            # ---------------- emit helpers ----------------
            def emit_scores_exp(b, kc):
                qs = slice(b * QB, (b + 1) * QB)
                ks = slice(kc * 128, (kc + 1) * 128)
                sc = next_slot()
                if FP8:
                    mm = [(sc[:, 0:256], qk_ap(kA, slice(0, 64), ks),
                           qk_ap(qA, slice(0, 64), qs), None),
                          (sc[:, 256:512], qk_ap(kA, slice(64, 128), ks),
                           qk_ap(qA, slice(64, 128), qs), (64, 0)),
                          (sc[:, 512:768], qk_ap(kB, slice(0, 64), ks),
                           qk_ap(qB, slice(0, 64), qs), None)]
                    for out_, l_, r_, tp in mm:
                        nc.tensor.matmul(out_, l_, r_, perf_mode=DR,
                                         start=True, stop=True,
                                         tile_position=tp,
                                         skip_group_check=True)
                else:
                    mm = [(sc[:, 0:256], kA[0:64, ks], qA[0:64, qs], None),
                          (sc[:, 256:512], kA[64:128, ks],
                           qA[64:128, qs], (64, 0)),
                          (sc[:, 512:768], kB[:, ks], qB[:, qs], None)]
                    for out_, l_, r_, tp in mm:
                        nc.tensor.matmul(out_, l_, r_, start=True, stop=True,
                                         tile_position=tp,
                                         skip_group_check=True)
                eng = EXP_PAT[(b * NKC + kc) % len(EXP_PAT)]
                if eng == "A":
                    e = expp.tile([128, 768], bf16, tag="ea", name="ea")
                    nc.scalar.activation(e, sc, AF.Exp,
                                         scale=0.125 / (4096.0 if XP8 else 1.0))
                else:
                    e = expp.tile([128, 768], f32, tag="eb", name="eb")
                    nc.vector.tensor_scalar(
                        e, sc, float(_AS / (4096.0 if XP8 else 1.0)),
                        float(_BS), ALU.mult, ALU.add)
                exp_tiles[(b, kc)] = (eng, e)

            def emit_attnv(b, kc, acc):
                eng, e = exp_tiles.pop((b, kc))
                if ABL_NOAV:
                    return
                if eng == "A":
                    full = e
                else:
                    full = e.bitcast(bf16).rearrange(
                        "p (c x) -> p c x", x=2)[:, :, 0]
                for h in range(HPC):
                    for q2 in range(2):
                        stat = full[:, h * 256 + q2 * 128:
                                    h * 256 + q2 * 128 + 128]
                        off = q2 * 195 + h * 65
                        nc.tensor.matmul(
                            acc[:, off:off + 65], stat,
                            vaug[:, kc * 195 + h * 65:kc * 195 + h * 65 + 65],
                            start=(kc == 0 and h == 0 and q2 == 0),
                            stop=(kc == NKC - 1 and h == HPC - 1 and q2 == 1),
                            skip_group_check=True)

            def emit_tail1(b, acc):
                """recip + stride-0-broadcast normalize (bf16) + XBAR."""
                if ABL_NOT1 or ABL_NOAV:
                    return
                for q2 in range(2):
                    off = q2 * 195
                    rc = attnp.tile([128, 4], f32, tag="rc", name="rc")
                    se = acc[:, off:off + 195].rearrange(
                        "p (c x) -> p c x", x=65)[:, :, 64]
                    nc.vector.reciprocal(rc[:, 0:3], se)
                    an = attnp.tile([128, 256], bf16, tag="an", name="an")
                    rcb = rc[:, 0:3].unsqueeze(2).broadcast_to([128, 3, 65])
                    nc.vector.tensor_tensor(
                        an[:, 0:195].rearrange("p (h c) -> p h c", h=3),
                        acc[:, off:off + 195].rearrange(
                            "p (h c) -> p h c", h=3),
                        rcb, ALU.mult)
                    nc.gpsimd.memset(an[:, 195:256], 0.0)
                    aT1 = attnp.tile([128, 128], bf16, tag="aT1", name="aT1")
                    aT2 = attnp.tile([128, 128], bf16, tag="aT2", name="aT2")
                    nc.sync.dma_start_transpose(aT1, an[:, 0:128])
                    nc.sync.dma_start_transpose(aT2, an[:, 128:256])
                    tailst[(b, q2)] = (aT1, aT2)

            def emit_tail2(b):
                """O-projection into a stolen score slot + store."""
                if ABL_NOT2 or ABL_NOT1 or ABL_NOAV:
                    return
                for q2 in range(2):
                    aT1, aT2 = tailst.pop((b, q2))
                    op = next_slot()
                    for third in range(3):
                        oc = slice(third * 256, third * 256 + 256)
                        nc.tensor.matmul(op[:, oc], aT1, wo1_t[:, oc],
                                         start=True, stop=False,
                                         skip_group_check=True)
                        nc.tensor.matmul(op[:, oc], aT2[0:67, :],
                                         wo2_t[:, oc], start=False, stop=True,
                                         skip_group_check=True)
                    ot = attnp.tile([128, D], f32, tag="ot", name="ot")
                    nc.scalar.activation(ot, op, AF.Copy)
                    r0 = b * QB + q2 * 128
                    nc.sync.dma_start(out=o[r0:r0 + 128, :], in_=ot)

            # ---------------- phase A (+ block-0 interleave) ----------------
            def emit_proj(t):
                tcols = slice(t * 512, (t + 1) * 512)
                for g in range(3):
                    pj = next_slot()
                    if XP8:
                        for j in range(3):
                            nc.tensor.matmul(
                                pj[:, 0:512], wqk8v[:, j, g],
                                x8t[j].rearrange(
                                    "p (jj c) -> p jj c", jj=2)[:, :, tcols],
                                perf_mode=DR, start=(j == 0), stop=(j == 2),
                                skip_group_check=True)
                    else:
                        for dc in range(6):
                            nc.tensor.matmul(
                                pj[:, 0:512],
                                wqk_t[:, dc * 384 + g * 128:
                                      dc * 384 + (g + 1) * 128],
                                xt[dc][:, tcols], start=(dc == 0),
                                stop=(dc == 5), skip_group_check=True)
                    if g == 0:
                        nc.scalar.activation(qA[:, tcols], pj[:, 0:512],
                                             AF.Identity,
                                             bias=bias_t[:, 0:1])
                    elif g == 1:
                        nc.scalar.activation(kA[:, tcols], pj[:, 0:512],
                                             AF.Identity,
                                             bias=bias_t[:, 1:2])
                    else:
                        nc.scalar.activation(qB[:, tcols], pj[0:64, 0:512],
                                             AF.Identity,
                                             bias=bias_t[0:64, 2:3])
                        nc.scalar.activation(k2s[64:128, tcols],
                                             pj[64:128, 0:512],
                                             AF.Identity,
                                             bias=bias_t[64:128, 2:3])
                if t % 2 == 1:
                    sh = slice((t - 1) * 512, (t + 1) * 512)
                    nc.sync.dma_start(out=kB[:, sh], in_=k2s[64:128, sh])
                for i in range(4):
                    kc = t * 4 + i
                    vp = next_slot()
                    for dc in range(6):
                        nc.tensor.matmul(
                            vp[:, 0:192],
                            xt[dc][:, kc * 128:(kc + 1) * 128],
                            wv_t[:, dc * 192:(dc + 1) * 192],
                            start=(dc == 0), stop=(dc == 5),
                            skip_group_check=True)
                    nc.vector.tensor_copy(
                        vaug4[:, kc, :, 0:64],
                        vp[:, 0:192].rearrange("p (h c) -> p h c", h=3))

            b0_acc = accp.tile([128, 390], f32, tag="acc", name="acc0")
            for t in range(8):
                emit_proj(t)
                if t >= 1:
                    for kc in range(4 * (t - 1), 4 * t):
                        emit_scores_exp(0, kc)
                        if kc >= LAG:
                            emit_attnv(0, kc - LAG, b0_acc)

            # ---------------- blocks ----------------
            carry = None
            cur = (0, b0_acc)
            for b in range(NQB):
                if b == 0:
                    for kc in range(28, 32):
                        emit_scores_exp(0, kc)
                        emit_attnv(0, kc - LAG, b0_acc)
                    for kc in range(NKC - LAG, NKC):
                        emit_attnv(0, kc, b0_acc)
                else:
                    acc = accp.tile([128, 390], f32, tag="acc", name="acc")
                    cur = (b, acc)
                    for kc in range(NKC):
                        emit_scores_exp(b, kc)
                        if kc >= LAG:
                            emit_attnv(b, kc - LAG, acc)
                        if kc == TAIL1_KC and carry is not None:
                            emit_tail1(carry[0], carry[1])
                        if kc == TAIL2_KC and carry is not None:
                            emit_tail2(carry[0])
                    for kc in range(NKC - LAG, NKC):
                        emit_attnv(b, kc, acc)
                carry = cur
            emit_tail1(carry[0], carry[1])
            emit_tail2(carry[0])

    nc.finalize()
    return nc


def _get_nc():
    if "nc" not in _cache:
        _cache["nc"] = _build_nc()
    return _cache["nc"]


def _make_in_maps(x, W_q, b_q, W_k, b_k, W_v, b_v, W_o, b_o):
    import ml_dtypes
    bf = ml_dtypes.bfloat16
    in_maps = []
    for c in range(N_CORES):
        b = c // 4
        h0 = (c % 4) * HPC
        c0 = h0 * DK

        g0 = W_q[:, c0:c0 + 128]
        g1 = W_k[:, c0:c0 + 128]
        g2 = np.concatenate([W_q[:, c0 + 128:c0 + 192],
                             W_k[:, c0 + 128:c0 + 192]], axis=1)
        wqk_m = np.concatenate([g0, g1, g2], axis=1)

        bpack = np.zeros((128, 3), np.float32)
        bpack[:, 0] = b_q[c0:c0 + 128]
        bpack[:, 1] = b_k[c0:c0 + 128]
        bpack[0:64, 2] = b_q[c0 + 128:c0 + 192]
        bpack[64:128, 2] = b_k[c0 + 128:c0 + 192]

        # woaug [195, 768]: per head rows 0..63 = W_o rows; row 64 = 0
        woaug = np.zeros((195, D), np.float32)
        for j in range(HPC):
            woaug[j * 65:j * 65 + 64, :] = \
                W_o[c0 + j * DK:c0 + (j + 1) * DK, :]

        fp8t = ml_dtypes.float8_e4m3fn
        xT = x[b].T
        x8m = np.concatenate(
            [np.concatenate([xT[256 * j:256 * j + 128, :],
                             xT[256 * j + 128:256 * j + 256, :]], axis=1)
             for j in range(3)], axis=0)
        wqk_s = wqk_m * WSC
        wqk8_m = np.zeros((128, 2304), np.float32)
        for j in range(3):
            for g in range(3):
                base = (j * 3 + g) * 256
                wqk8_m[:, base:base + 128] = \
                    wqk_s[256 * j:256 * j + 128, g * 128:(g + 1) * 128]
                wqk8_m[:, base + 128:base + 256] = \
                    wqk_s[256 * j + 128:256 * j + 256, g * 128:(g + 1) * 128]
        if XP8:
            bpack = bpack * (WSC * WSC)
        in_maps.append({
            "xbT": np.ascontiguousarray(x[b].T).astype(bf),
            "x8": (x8m * WSC).astype(fp8t),
            "wqk8": wqk8_m.astype(fp8t),
            "wqk": np.ascontiguousarray(wqk_m).astype(bf),
            "wv": np.ascontiguousarray(W_v[:, c0:c0 + 192]).astype(bf),
            "wo1": np.ascontiguousarray(woaug[0:128, :]).astype(bf),
            "wo2": np.ascontiguousarray(woaug[128:195, :]).astype(bf),
            "bpack": bpack,
        })
    return in_maps


def kernel(**inputs):
    from concourse.bass_utils import run_bass_kernel_spmd

    args = {k: np.asarray(v, dtype=np.float32) for k, v in inputs.items()}
    in_maps = _make_in_maps(
        args["x"], args["W_q"], args["b_q"], args["W_k"], args["b_k"],
        args["W_v"], args["b_v"], args["W_o"], args["b_o"])

    nc = _get_nc()
    trace = bool(int(os.environ.get("KBENCH_TRACE", "0")))
    res = run_bass_kernel_spmd(nc, in_maps, core_ids=list(range(N_CORES)),
                               trace=trace)
    _cache["last_result"] = res

    out = np.zeros((B, T, D), np.float32)
    for c in range(N_CORES):
        out[c // 4] += res.results[c]["o"]
    # bias constants folded on host: b_o plus every head's b_v @ W_o
    bias_row = args["b_o"] + args["b_v"] @ args["W_o"]
    out += bias_row[None, None, :]
    return out
